# revision 1
# baseline (speedup 1.0000x reference)
"""Trainium2 Bass kernel for a 12-layer BERT encoder forward pass.

Strategy: data-parallel over the batch across 8 NeuronCores (2 sequences each).
Each core runs the full encoder on its shard; no collectives. Activations are
kept feature-major on-chip; attention uses a transposed-scores layout with a
ones-column appended to V so the softmax denominator falls out of the AV
matmul. Matmuls run in float32r (4x fp32 throughput on the PE).
kernel(**inputs) takes the full inputs and returns the full [16,512,768] output.
"""
import sys
for _p in ('/opt/trn_rl_repo', '/root/.axon_site/_ro/trn_rl_repo'):
    if _p not in sys.path:
        sys.path.append(_p)
import numpy as np
from contextlib import ExitStack

import concourse.bass as bass
from concourse import bacc
import concourse.mybir as mybir
import concourse.tile as tile
from concourse.masks import make_identity
from concourse import tile_utils

# allow using the full usable SBUF (stale default is 192KB/partition)
tile_utils.max_sbuf_usage = 208 * 1024

f32 = mybir.dt.float32
f32r = mybir.dt.float32r
i32 = mybir.dt.int32
AF = mybir.ActivationFunctionType
ALU = mybir.AluOpType

P = 128
D = 768
KC = 6          # D / P
H = 12
HD = 64         # head dim
F = 3072
FC = 24         # F / P
S = 512
N = 1024        # tokens per core (2 seqs)
NT = 8          # N / P
EPS = 1e-12


class Ctx:
    pass


def build_nc(L=12, use_f32r=True, gelu_sim=False, reps=1):
    mmdt = f32r if use_f32r else f32
    g = Ctx()
    g.adt = mmdt            # dtype for tiles consumed by matmuls
    g.mc = lambda ap: ap    # matmul operands already carry the right dtype
    g.act_fn = AF.Tanh if gelu_sim else AF.Gelu

    nc = bacc.Bacc("TRN2", num_devices=8, dynamic_dma_scratch_size=4096)
    g.nc = nc

    # ---- DRAM inputs ----
    g.ids = nc.dram_tensor("ids", [N, 1], i32, kind="ExternalInput")
    g.word_emb = nc.dram_tensor("word_emb", [30522, D], f32, kind="ExternalInput")
    g.pos_type = nc.dram_tensor("pos_type", [S, D], f32, kind="ExternalInput")
    g.emb_w = nc.dram_tensor("emb_w", [1, D], f32, kind="ExternalInput")
    g.emb_b = nc.dram_tensor("emb_b", [1, D], f32, kind="ExternalInput")
    g.Wq = nc.dram_tensor("Wq", [L, D, D], mmdt, kind="ExternalInput")
    g.Wk = nc.dram_tensor("Wk", [L, D, D], mmdt, kind="ExternalInput")
    g.Wv = nc.dram_tensor("Wv", [L, D, D], mmdt, kind="ExternalInput")
    g.Wo = nc.dram_tensor("Wo", [L, D, D], mmdt, kind="ExternalInput")
    g.Wf1 = nc.dram_tensor("Wf1", [L, D, F], mmdt, kind="ExternalInput")
    g.Wf2 = nc.dram_tensor("Wf2", [L, F, D], mmdt, kind="ExternalInput")
    g.bqr = nc.dram_tensor("bqr", [L, P, KC], f32, kind="ExternalInput")
    g.bkr = nc.dram_tensor("bkr", [L, P, KC], f32, kind="ExternalInput")
    g.bv_row = nc.dram_tensor("bv_row", [L, 1, D], f32, kind="ExternalInput")
    g.bor = nc.dram_tensor("bor", [L, P, KC], f32, kind="ExternalInput")
    g.bf1r = nc.dram_tensor("bf1r", [L, P, FC], f32, kind="ExternalInput")
    g.bf2r = nc.dram_tensor("bf2r", [L, P, KC], f32, kind="ExternalInput")
    g.ln1wr = nc.dram_tensor("ln1wr", [L, P, KC], f32, kind="ExternalInput")
    g.ln1br = nc.dram_tensor("ln1br", [L, P, KC], f32, kind="ExternalInput")
    g.ln2wr = nc.dram_tensor("ln2wr", [L, P, KC], f32, kind="ExternalInput")
    g.ln2br = nc.dram_tensor("ln2br", [L, P, KC], f32, kind="ExternalInput")
    g.onesd = nc.dram_tensor("onesd", [P, 1], mmdt, kind="ExternalInput")
    g.out_fm = nc.dram_tensor("out_fm", [KC, P, N], f32, kind="ExternalOutput")

    with TileContextPools(g) as g:
        if reps > 1:
            with g.tc.For_i(0, reps, 1):
                _emit(g, L)
        else:
            _emit(g, L)

    nc.finalize()
    return nc


class TileContextPools:
    def __init__(self, g):
        self.g = g

    def __enter__(self):
        g = self.g
        self.stack = ExitStack()
        tc = self.stack.enter_context(tile.TileContext(g.nc))
        ep = self.stack.enter_context
        g.tc = tc
        g.act = ep(tc.tile_pool(name="act", bufs=3))
        g.ffp = ep(tc.tile_pool(name="ffp", bufs=1))
        g.vp = ep(tc.tile_pool(name="vp", bufs=1))
        g.wp = ep(tc.tile_pool(name="wp", bufs=2))
        g.qmp = ep(tc.tile_pool(name="qmp", bufs=2))
        g.expp = ep(tc.tile_pool(name="exp", bufs=2))
        g.bb = ep(tc.tile_pool(name="bb", bufs=2))
        g.dvp = ep(tc.tile_pool(name="dv", bufs=2))
        g.rowp = ep(tc.tile_pool(name="rowp", bufs=1))
        g.rowsp = ep(tc.tile_pool(name="rows", bufs=1))
        g.singles = ep(tc.tile_pool(name="singles", bufs=1))
        g.small = ep(tc.tile_pool(name="small", bufs=2))
        g.biasp = ep(tc.tile_pool(name="bias", bufs=2))
        g.pp = ep(tc.tile_pool(name="pp", bufs=2, space="PSUM"))
        g.scp = ep(tc.tile_pool(name="scp", bufs=2, space="PSUM"))
        g.avp = ep(tc.tile_pool(name="avp", bufs=2, space="PSUM"))
        return g

    def __exit__(self, *a):
        return self.stack.__exit__(*a)


def _emit(g, L):
    nc = g.nc
    g.ident = g.singles.tile([P, P], f32, tag="ident")
    make_identity(nc, g.ident[:])
    g.ones = g.singles.tile([P, 1], g.adt, tag="ones")
    nc.sync.dma_start(out=g.ones[:], in_=g.onesd[:])
    g.epsT = g.singles.tile([P, 1], f32, tag="eps")
    nc.vector.memset(g.epsT[:], EPS)

    hT = _embedding(g)
    for l in range(L):
        hT = _layer(g, l, hT)
    for k in range(KC):
        nc.sync.dma_start(out=g.out_fm[k], in_=hT[:, k, :].bitcast(f32))


def _embedding(g):
    nc = g.nc
    lnw_b = g.bb.tile([P, D], f32, tag="bb")
    lnb_b = g.bb.tile([P, D], f32, tag="bb")
    embwb = g.rowp.tile([1, 2, D], f32, tag="row")
    nc.sync.dma_start(out=embwb[:, 0, :], in_=g.emb_w[:])
    nc.gpsimd.partition_broadcast(lnw_b[:], embwb[:, 0, :])
    nc.sync.dma_start(out=embwb[:, 1, :], in_=g.emb_b[:])
    nc.gpsimd.partition_broadcast(lnb_b[:], embwb[:, 1, :])

    htok = g.act.tile([P, NT, D], f32, tag="act")
    for tt in range(NT):
        _embed_tile(g, htok, tt, lnw_b, lnb_b)

    hT = g.act.tile([P, KC, N], g.adt, tag="act")
    for k in range(KC):
        for tt in range(NT):
            ps = g.pp.tile([P, 512], f32, tag="pp")
            nc.tensor.transpose(ps[:, 0:P], htok[:, tt, k * P:(k + 1) * P], g.ident[:])
            nc.vector.tensor_copy(hT[:, k, tt * P:(tt + 1) * P], ps[:, 0:P])
    return hT


def _embed_tile(g, htok, tt, lnw_b, lnb_b):
    nc = g.nc
    idx = g.small.tile([P, 1], i32, tag="idx")
    nc.sync.dma_start(out=idx[:], in_=g.ids[tt * P:(tt + 1) * P, :])
    gt = g.wp.tile([P, D], f32, tag="w")
    nc.gpsimd.indirect_dma_start(
        out=gt[:], out_offset=None, in_=g.word_emb[:],
        in_offset=bass.IndirectOffsetOnAxis(ap=idx[:, :1], axis=0),
    )
    pt = g.wp.tile([P, D], f32, tag="w")
    nc.sync.dma_start(out=pt[:], in_=g.pos_type[(tt % 4) * P:(tt % 4 + 1) * P, :])
    nc.vector.tensor_add(htok[:, tt, :], gt[:], pt[:])
    xr = htok[:, tt, :].rearrange("p (s f) -> p s f", f=256)
    stats = g.small.tile([P, 3, 6], f32, tag="bnst")
    for sgi in range(3):
        nc.vector.bn_stats(out=stats[:, sgi, :], in_=xr[:, sgi, :])
    mv = g.small.tile([P, 2], f32, tag="bnmv")
    nc.vector.bn_aggr(out=mv[:], in_=stats[:])
    sd = g.small.tile([P, 1], f32, tag="sd")
    nc.scalar.activation(sd[:], mv[:, 1:2], AF.Sqrt, bias=g.epsT[:, 0:1], scale=1.0)
    nc.vector.reciprocal(sd[:], sd[:])
    nc.vector.tensor_scalar(
        out=htok[:, tt, :], in0=htok[:, tt, :],
        scalar1=mv[:, 0:1], scalar2=sd[:, 0:1],
        op0=ALU.subtract, op1=ALU.mult,
    )
    nc.vector.tensor_mul(htok[:, tt, :], htok[:, tt, :], lnw_b[:])
    nc.vector.tensor_add(htok[:, tt, :], htok[:, tt, :], lnb_b[:])


def _layer(g, l, hT):
    nc, mc = g.nc, g.mc
    bias = g.biasp.tile([P, 8, KC], f32, tag="b6", name=f"bias{l}")
    for j, dram in enumerate((g.bqr, g.bkr, g.bor, g.bf2r,
                              g.ln1wr, g.ln1br, g.ln2wr, g.ln2br)):
        nc.sync.dma_start(out=bias[:, j, :], in_=dram[l])
    bqt, bkt, bot, bf2t = (bias[:, j, :] for j in range(4))
    l1w, l1b, l2w, l2b = (bias[:, j, :] for j in range(4, 8))
    bf1t = g.biasp.tile([P, FC], f32, tag="b24")
    nc.sync.dma_start(out=bf1t[:], in_=g.bf1r[l])
    bvrow = g.rowp.tile([1, D], f32, tag="row")
    nc.sync.dma_start(out=bvrow[:], in_=g.bv_row[l])
    bvb = g.bb.tile([P, D], f32, tag="bb")
    nc.gpsimd.partition_broadcast(bvb[:], bvrow[:])

    # ---- K projection (feature-major) ----
    kT = g.act.tile([P, KC, N], g.adt, tag="act")
    _proj_fm(g, g.Wk[l], hT, kT, bkt)

    # ---- V projection (token-major + ones col) ----
    v = _vproj(g, l, hT, bvb)

    # ---- attention ----
    aT = g.act.tile([P, KC, N], g.adt, tag="act")
    for mcb in range(KC):
        _attn_block(g, l, mcb, hT, kT, v, aT, bqt)

    # ---- O projection + residual + LN1 in-place (c-outer) ----
    x = g.act.tile([P, KC, N], g.adt, tag="act")
    wov = g.Wo[l].rearrange("(kc p) n -> p kc n", p=P)
    for c in range(2):
        for m in range(KC):
            wmb = g.wp.tile([P, KC, P], g.adt, tag="w")
            nc.sync.dma_start(out=wmb[:], in_=wov[:, :, m * P:(m + 1) * P])
            ps = g.pp.tile([P, 512], f32, tag="pp")
            for k in range(KC):
                nc.tensor.matmul(
                    ps[:], lhsT=mc(wmb[:, k, :]),
                    rhs=mc(aT[:, k, c * 512:(c + 1) * 512]),
                    start=(k == 0), stop=(k == KC - 1),
                )
            nc.vector.tensor_scalar_add(ps[:], ps[:], bot[:, m:m + 1])
            nc.vector.tensor_tensor(
                out=x[:, m, c * 512:(c + 1) * 512], in0=ps[:],
                in1=hT[:, m, c * 512:(c + 1) * 512], op=ALU.add)
        _ln_chunk(g, x, l1w, l1b, c)
    h1 = x  # normalized in place

    # ---- FFN (token-chunked, c-outer) + LN2 in-place ----
    x2 = g.act.tile([P, KC, N], g.adt, tag="act")
    for c in range(2):
        _ffn_chunk(g, l, c, h1, x2, bf1t, bf2t)
        _ln_chunk(g, x2, l2w, l2b, c)
    return x2


def _proj_fm(g, Wdram, hT, outT, bias_t):
    """Feature-major projection: outT[m, t] = W.T @ hT + bias."""
    nc, mc = g.nc, g.mc
    wview = Wdram.rearrange("(kc p) n -> p kc n", p=P)
    for m in range(KC):
        wmb = g.wp.tile([P, KC, P], g.adt, tag="w")
        nc.sync.dma_start(out=wmb[:], in_=wview[:, :, m * P:(m + 1) * P])
        for c in range(2):
            ps = g.pp.tile([P, 512], f32, tag="pp")
            for k in range(KC):
                nc.tensor.matmul(
                    ps[:], lhsT=mc(wmb[:, k, :]),
                    rhs=mc(hT[:, k, c * 512:(c + 1) * 512]),
                    start=(k == 0), stop=(k == KC - 1),
                )
            nc.vector.tensor_scalar_add(
                outT[:, m, c * 512:(c + 1) * 512], ps[:], bias_t[:, m:m + 1])


def _vproj(g, l, hT, bvb):
    nc, mc = g.nc, g.mc
    wv = g.ffp.tile([P, KC, D], g.adt, tag="ff")
    nc.sync.dma_start(out=wv[:], in_=g.Wv[l].rearrange("(kc p) n -> p kc n", p=P))
    v = g.vp.tile([P, H, NT, HD + 1], g.adt, tag="v")
    nc.vector.tensor_copy(v[:, :, :, HD:HD + 1],
                          g.ones[:].to_broadcast((P, H, NT, 1)))
    for tt in range(NT):
        for (cs, cl) in ((0, 512), (512, 256)):
            ps = g.pp.tile([P, 512], f32, tag="pp")
            for k in range(KC):
                nc.tensor.matmul(
                    ps[:, :cl], lhsT=mc(hT[:, k, tt * P:(tt + 1) * P]),
                    rhs=mc(wv[:, k, cs:cs + cl]),
                    start=(k == 0), stop=(k == KC - 1),
                )
            nh = cl // HD
            h0 = cs // HD
            nc.vector.tensor_tensor(
                out=v[:, h0:h0 + nh, tt, 0:HD],
                in0=ps[:, :cl].rearrange("p (h d) -> p h d", d=HD),
                in1=bvb[:, cs:cs + cl].rearrange("p (h d) -> p h d", d=HD),
                op=ALU.add,
            )
    return v


def _attn_block(g, l, mcb, hT, kT, v, aT, bqt):
    """Q projection for feature block mcb (heads 2*mcb, 2*mcb+1) then attention."""
    nc, mc = g.nc, g.mc
    qm = g.qmp.tile([P, N], g.adt, tag="qm")
    wqv = g.Wq[l].rearrange("(kc p) n -> p kc n", p=P)
    wmb = g.wp.tile([P, KC, P], g.adt, tag="w")
    nc.sync.dma_start(out=wmb[:], in_=wqv[:, :, mcb * P:(mcb + 1) * P])
    for c in range(2):
        ps = g.pp.tile([P, 512], f32, tag="pp")
        for k in range(KC):
            nc.tensor.matmul(
                ps[:], lhsT=mc(wmb[:, k, :]),
                rhs=mc(hT[:, k, c * 512:(c + 1) * 512]),
                start=(k == 0), stop=(k == KC - 1),
            )
        nc.vector.tensor_scalar_add(
            qm[:, c * 512:(c + 1) * 512], ps[:], bqt[:, mcb:mcb + 1])
    for hh in range(2):
        for s in range(2):
            _attn_head_seq(g, mcb, hh, s, kT, qm, v, aT)


def _attn_head_seq(g, mcb, hh, s, kT, qm, v, aT):
    nc, mc = g.nc, g.mc
    h = 2 * mcb + hh
    et = [None, None]
    for half in range(2):
        sc = g.scp.tile([P, 2, S], f32, tag="sc")
        et[half] = g.expp.tile([P, 2, S], g.adt, tag="exp", name=f"et{half}")
        for i in range(2):
            ck = half * 2 + i
            nc.tensor.matmul(
                sc[:, i, :],
                lhsT=mc(kT[hh * HD:(hh + 1) * HD, mcb,
                           s * S + ck * P:s * S + (ck + 1) * P]),
                rhs=mc(qm[hh * HD:(hh + 1) * HD, s * S:(s + 1) * S]),
                start=True, stop=True,
            )
            nc.scalar.activation(et[half][:, i, :], sc[:, i, :], AF.Exp, scale=0.125)
    av = g.avp.tile([P, S], f32, tag="av")
    for ck in range(4):
        nc.tensor.matmul(
            av[0:HD + 1, :],
            lhsT=mc(v[:, h, s * 4 + ck, :]),
            rhs=mc(et[ck // 2][:, ck % 2, :]),
            start=(ck == 0), stop=(ck == 3),
        )
    dinv = g.small.tile([1, S], f32, tag="dinv")
    nc.vector.reciprocal(dinv[:], av[HD:HD + 1, :])
    dib = g.dvp.tile([HD, S], f32, tag="dv")
    nc.gpsimd.partition_broadcast(dib[:], dinv[:])
    nc.vector.tensor_tensor(
        out=aT[hh * HD:(hh + 1) * HD, mcb, s * S:(s + 1) * S],
        in0=av[0:HD, :], in1=dib[:], op=ALU.mult,
    )


def _ffn_chunk(g, l, c, h1, x2, bf1t, bf2t):
    nc, mc = g.nc, g.mc
    f1v = g.Wf1[l].rearrange("(kc p) n -> p kc n", p=P)
    f2v = g.Wf2[l].rearrange("(kc p) n -> p kc n", p=P)
    ffT = g.ffp.tile([P, FC, 512], g.adt, tag="ff")
    for m in range(FC):
        wmb = g.wp.tile([P, KC, P], g.adt, tag="w")
        nc.sync.dma_start(out=wmb[:], in_=f1v[:, :, m * P:(m + 1) * P])
        ps = g.pp.tile([P, 512], f32, tag="pp")
        for k in range(KC):
            nc.tensor.matmul(
                ps[:], lhsT=mc(wmb[:, k, :]),
                rhs=mc(h1[:, k, c * 512:(c + 1) * 512]),
                start=(k == 0), stop=(k == KC - 1),
            )
        nc.scalar.activation(
            ffT[:, m, :], ps[:], g.act_fn, bias=bf1t[:, m:m + 1], scale=1.0)
    for m in range(KC):
        ps = g.pp.tile([P, 512], f32, tag="pp")
        for khalf in range(2):
            wmb2 = g.wp.tile([P, 12, P], g.adt, tag="w")
            nc.sync.dma_start(
                out=wmb2[:],
                in_=f2v[:, khalf * 12:(khalf + 1) * 12, m * P:(m + 1) * P])
            for kk in range(12):
                k = khalf * 12 + kk
                nc.tensor.matmul(
                    ps[:], lhsT=mc(wmb2[:, kk, :]),
                    rhs=mc(ffT[:, k, :]),
                    start=(k == 0), stop=(k == FC - 1),
                )
        nc.vector.tensor_scalar_add(ps[:], ps[:], bf2t[:, m:m + 1])
        nc.vector.tensor_tensor(
            out=x2[:, m, c * 512:(c + 1) * 512], in0=ps[:],
            in1=h1[:, m, c * 512:(c + 1) * 512], op=ALU.add)


def _ln_chunk(g, x, w, b, c):
    """In-place LayerNorm of feature-major x for token chunk c."""
    nc, mc = g.nc, g.mc
    cs = slice(c * 512, (c + 1) * 512)
    rows = g.rowsp.tile([1, 3, 512], f32, tag="rows")
    mrow, qrow, m2 = rows[:, 0, :], rows[:, 1, :], rows[:, 2, :]
    pS = g.pp.tile([1, 512], f32, tag="pp")
    for k in range(KC):
        nc.tensor.matmul(pS[:], lhsT=mc(g.ones[:]), rhs=mc(x[:, k, cs]),
                         start=(k == 0), stop=(k == KC - 1))
    nc.vector.tensor_scalar_mul(mrow, pS[:], 1.0 / D)
    pQ = g.pp.tile([1, 512], f32, tag="pp")
    for k in range(KC):
        sq = g.qmp.tile([P, 512], g.adt, tag="qm")
        nc.vector.tensor_mul(sq[:], x[:, k, cs], x[:, k, cs])
        nc.tensor.matmul(pQ[:], lhsT=mc(g.ones[:]), rhs=mc(sq[:]),
                         start=(k == 0), stop=(k == KC - 1))
    nc.vector.tensor_scalar_mul(qrow, pQ[:], 1.0 / D)
    # var = E[x^2] - mean^2 ; A = 1/sqrt(var+eps) ; B = -mean*A
    nc.vector.tensor_mul(m2, mrow, mrow)
    nc.vector.tensor_tensor(out=qrow, in0=qrow, in1=m2, op=ALU.subtract)
    nc.scalar.activation(qrow, qrow, AF.Sqrt, bias=g.epsT[0:1, 0:1], scale=1.0)
    nc.vector.reciprocal(qrow, qrow)
    nc.vector.tensor_mul(m2, mrow, qrow)
    nc.vector.tensor_scalar_mul(m2, m2, -1.0)
    A = g.qmp.tile([P, 512], f32, tag="qm", name="lnA")
    B = g.qmp.tile([P, 512], f32, tag="qm", name="lnB")
    nc.gpsimd.partition_broadcast(A[:], qrow)
    nc.gpsimd.partition_broadcast(B[:], m2)
    for k in range(KC):
        nc.vector.tensor_mul(x[:, k, cs], x[:, k, cs], A[:])
        nc.vector.tensor_add(x[:, k, cs], x[:, k, cs], B[:])
        nc.gpsimd.tensor_scalar(
            out=x[:, k, cs], in0=x[:, k, cs],
            scalar1=w[:, k:k + 1], scalar2=b[:, k:k + 1],
            op0=ALU.mult, op1=ALU.add)


# ======================= host-side prep / sharding =======================



def _r6(a, L, nchunk):
    # [L, D_or_F] -> [L, P, nchunk] with feature f = k*128 + p
    Ld = a[:L]
    return np.ascontiguousarray(
        Ld.reshape(L, nchunk, P).transpose(0, 2, 1)).astype(np.float32)


def prep_shared(inputs, L=12):
    f = lambda x: np.ascontiguousarray(np.asarray(x, dtype=np.float32))
    w = {
        "word_emb": f(inputs["word_emb"]),
        "pos_type": f(np.asarray(inputs["pos_emb"])[:S] +
                      np.asarray(inputs["type_emb"])[0][None, :]),
        "emb_w": f(inputs["emb_ln_w"]).reshape(1, D),
        "emb_b": f(inputs["emb_ln_b"]).reshape(1, D),
        "Wq": f(inputs["Wq"][:L]), "Wk": f(inputs["Wk"][:L]),
        "Wv": f(inputs["Wv"][:L]), "Wo": f(inputs["Wo"][:L]),
        "Wf1": f(inputs["Wf1"][:L]), "Wf2": f(inputs["Wf2"][:L]),
        "bqr": _r6(np.asarray(inputs["bq"]), L, KC),
        "bkr": _r6(np.asarray(inputs["bk"]), L, KC),
        "bv_row": f(inputs["bv"][:L]).reshape(L, 1, D),
        "bor": _r6(np.asarray(inputs["bo"]), L, KC),
        "bf1r": _r6(np.asarray(inputs["bf1"]), L, FC),
        "bf2r": _r6(np.asarray(inputs["bf2"]), L, KC),
        "ln1wr": _r6(np.asarray(inputs["ln1_w"]), L, KC),
        "ln1br": _r6(np.asarray(inputs["ln1_b"]), L, KC),
        "ln2wr": _r6(np.asarray(inputs["ln2_w"]), L, KC),
        "ln2br": _r6(np.asarray(inputs["ln2_b"]), L, KC),
        "onesd": np.ones((P, 1), np.float32),
    }
    return w


def core_ids_input(input_ids, core):
    return np.ascontiguousarray(
        np.asarray(input_ids)[2 * core:2 * core + 2].reshape(N, 1)).astype(np.int32)


def assemble_output(out_fm):
    # [KC, P, N] feature-major -> [2, S, D] token-major
    return np.ascontiguousarray(out_fm.reshape(D, N).T).reshape(2, S, D)


_NC_CACHE = {}


def kernel(**inputs):
    from concourse.bass_utils import run_bass_kernel_spmd

    am = np.asarray(inputs["attention_mask"])
    assert (am == 1).all(), "kernel specialized for all-ones attention_mask"

    if "nc" not in _NC_CACHE:
        _NC_CACHE["nc"] = build_nc(L=12, use_f32r=True)
    nc = _NC_CACHE["nc"]

    shared = prep_shared(inputs, L=12)
    in_maps = []
    for core in range(8):
        m = dict(shared)
        m["ids"] = core_ids_input(inputs["input_ids"], core)
        in_maps.append(m)

    res = run_bass_kernel_spmd(nc, in_maps, list(range(8)), trace=False)
    out = np.concatenate(
        [assemble_output(res.results[c]["out_fm"]) for c in range(8)], axis=0)
    return out.astype(np.float32)



# revision 25
# speedup vs baseline: 22.8128x; 22.8128x over previous
"""Trainium2 Bass kernel for a 12-layer BERT encoder forward pass.

Strategy: data-parallel over the batch across 8 NeuronCores (2 sequences each).
Each core runs the full encoder on its shard; no collectives.

Precision: matmuls run in float32r (4x fp32 PE throughput, fp32-grade
accuracy); the residual/LayerNorm stream is fp32. Only the attention
probabilities (exp of scores) and V are bf16 — their products accumulate in
fp32 PSUM, adding a few 1e-3 of output noise while halving their SBUF cost.

Layout: activations are feature-major [128 feat x tokens] on-chip; attention
uses a transposed-scores layout with a ones-column appended to V so the
softmax denominator falls out of the AV matmul; the K projection is fully
weight-preloaded and runs token-chunk outer, interleaved with V projection
tiles, so the previous LayerNorm tail hides under matmuls.
kernel(**inputs) takes the full inputs and returns the full [16,512,768] output.
"""
import sys
for _p in ('/opt/trn_rl_repo', '/root/.axon_site/_ro/trn_rl_repo'):
    if _p not in sys.path:
        sys.path.append(_p)
import numpy as np
from contextlib import ExitStack

import concourse.bass as bass
from concourse import bacc
import concourse.mybir as mybir
import concourse.tile as tile
from concourse.masks import make_identity
from concourse import tile_utils

# allow using the full usable SBUF (stale default is 192KB/partition)
tile_utils.max_sbuf_usage = 208 * 1024

f32 = mybir.dt.float32
f32r = mybir.dt.float32r
bf16 = mybir.dt.bfloat16
i32 = mybir.dt.int32
AF = mybir.ActivationFunctionType
ALU = mybir.AluOpType

P = 128
D = 768
KC = 6          # D / P
H = 12
HD = 64         # head dim
F = 3072
FC = 24         # F / P
S = 512
N = 1024        # tokens per core (2 seqs)
NT = 8          # N / P
EPS = 1e-12


class Ctx:
    pass


def build_nc(L=12, use_f32r=True, gelu_sim=False, reps=1, emit_layers=None,
             zero_bias=False):
    del use_f32r
    g = Ctx()
    g.adt = f32r            # matmul activation dtype
    g.pdt = bf16            # attention probabilities / V dtype
    g.act_fn = AF.Tanh if gelu_sim else AF.Gelu
    g.L = L
    g.emit_layers = L if emit_layers is None else emit_layers
    g.zb = zero_bias        # skip structurally-zero bias / identity-LN ops

    nc = bacc.Bacc("TRN2", num_devices=8, dynamic_dma_scratch_size=4096)
    g.nc = nc

    # ---- DRAM inputs ----
    g.ids = nc.dram_tensor("ids", [N, 1], i32, kind="ExternalInput")
    g.word_emb = nc.dram_tensor("word_emb", [30522, D], f32, kind="ExternalInput")
    g.pos_type = nc.dram_tensor("pos_type", [S, D], f32, kind="ExternalInput")
    g.emb_w = nc.dram_tensor("emb_w", [1, D], f32, kind="ExternalInput")
    g.emb_b = nc.dram_tensor("emb_b", [1, D], f32, kind="ExternalInput")
    g.Wq = nc.dram_tensor("Wq", [L, D, D], f32r, kind="ExternalInput")
    g.Wk = nc.dram_tensor("Wk", [L, D, D], f32r, kind="ExternalInput")
    g.Wv = nc.dram_tensor("Wv", [L, D, D], f32r, kind="ExternalInput")
    g.Wo = nc.dram_tensor("Wo", [L, D, D], f32r, kind="ExternalInput")
    g.Wf1 = nc.dram_tensor("Wf1", [L, D, F], f32r, kind="ExternalInput")
    g.Wf2 = nc.dram_tensor("Wf2", [L, F, D], f32r, kind="ExternalInput")
    g.bqr = nc.dram_tensor("bqr", [L, P, KC], f32, kind="ExternalInput")
    g.bkr = nc.dram_tensor("bkr", [L, P, KC], f32, kind="ExternalInput")
    g.bv_row = nc.dram_tensor("bv_row", [L, 1, D], f32, kind="ExternalInput")
    g.bor = nc.dram_tensor("bor", [L, P, KC], f32, kind="ExternalInput")
    g.bf1r = nc.dram_tensor("bf1r", [L, P, FC], f32, kind="ExternalInput")
    g.bf2r = nc.dram_tensor("bf2r", [L, P, KC], f32, kind="ExternalInput")
    g.ln1wr = nc.dram_tensor("ln1wr", [L, P, KC], f32, kind="ExternalInput")
    g.ln1br = nc.dram_tensor("ln1br", [L, P, KC], f32, kind="ExternalInput")
    g.ln2wr = nc.dram_tensor("ln2wr", [L, P, KC], f32, kind="ExternalInput")
    g.ln2br = nc.dram_tensor("ln2br", [L, P, KC], f32, kind="ExternalInput")
    g.onesd = nc.dram_tensor("onesd", [P, 1], bf16, kind="ExternalInput")
    g.onesrd = nc.dram_tensor("onesrd", [P, 1], f32r, kind="ExternalInput")
    g.out_fm = nc.dram_tensor("out_fm", [KC, P, N], f32, kind="ExternalOutput")

    with TileContextPools(g) as g:
        if reps > 1:
            with g.tc.For_i(0, reps, 1):
                _emit(g, L)
        else:
            _emit(g, L)

    nc.finalize()
    return nc


class TileContextPools:
    def __init__(self, g):
        self.g = g

    def __enter__(self):
        g = self.g
        self.stack = ExitStack()
        tc = self.stack.enter_context(tile.TileContext(g.nc))
        ep = self.stack.enter_context
        g.tc = tc
        g.act = ep(tc.tile_pool(name="act", bufs=3))     # f32r activations
        g.ffp = ep(tc.tile_pool(name="ffp", bufs=1))     # gelu outputs (f32r)
        g.vp = ep(tc.tile_pool(name="vp", bufs=1))       # V (bf16)
        g.wfp = ep(tc.tile_pool(name="wfp", bufs=1))     # Wk full preload
        g.wvp = ep(tc.tile_pool(name="wvp", bufs=1))     # Wv full load
        g.wp = ep(tc.tile_pool(name="wp", bufs=2))
        g.qmp = ep(tc.tile_pool(name="qmp", bufs=3))
        g.expp = ep(tc.tile_pool(name="exp", bufs=2))    # exp tiles (bf16)
        g.bb = ep(tc.tile_pool(name="bb", bufs=2))
        g.dvp = ep(tc.tile_pool(name="dv", bufs=2))
        g.emb = ep(tc.tile_pool(name="emb", bufs=1))
        g.rowp = ep(tc.tile_pool(name="rowp", bufs=1))
        g.rowsp = ep(tc.tile_pool(name="rows", bufs=1))
        g.singles = ep(tc.tile_pool(name="singles", bufs=1))
        g.small = ep(tc.tile_pool(name="small", bufs=1))
        g.biasp = ep(tc.tile_pool(name="bias", bufs=2))
        g.pp = ep(tc.tile_pool(name="pp", bufs=3, space="PSUM"))
        g.scp = ep(tc.tile_pool(name="scp", bufs=2, space="PSUM"))
        g.avp = ep(tc.tile_pool(name="avp", bufs=2, space="PSUM"))
        g.lnp = ep(tc.tile_pool(name="lnp", bufs=1, space="PSUM"))
        return g

    def __exit__(self, *a):
        return self.stack.__exit__(*a)


def _emit(g, L):
    nc = g.nc
    if g.emit_layers == 0 and L > 0:
        # control program for timing: same I/O, near-zero device work
        z = g.small.tile([P, N], f32, tag="zed")
        nc.vector.memset(z[:], 0.0)
        for k in range(KC):
            nc.sync.dma_start(out=g.out_fm[k], in_=z[:])
        return
    g.ident = g.singles.tile([P, P], f32, tag="ident")
    make_identity(nc, g.ident[:])
    g.ones = g.singles.tile([P, 1], bf16, tag="ones")
    nc.sync.dma_start(out=g.ones[:], in_=g.onesd[:])
    g.onesr = g.singles.tile([P, 1], f32r, tag="onesr")
    nc.sync.dma_start(out=g.onesr[:], in_=g.onesrd[:])
    g.epsT = g.singles.tile([P, 1], f32, tag="eps")
    nc.vector.memset(g.epsT[:], EPS)

    hT = _embedding(g)
    wk = g.wfp.tile([P, KC, D], f32r, tag="wfull", name="wk0")
    nc.sync.dma_start(out=wk[:], in_=g.Wk[0].rearrange("(kc p) n -> p kc n", p=P))
    g.L = g.emit_layers
    for l in range(g.emit_layers):
        hT, wk = _layer(g, l, hT, wk)
    for k in range(KC):
        nc.sync.dma_start(out=g.out_fm[k], in_=hT[:, k, :].bitcast(f32))


def _embedding(g):
    nc = g.nc
    lnw_b = g.bb.tile([P, D], f32, tag="bb")
    lnb_b = g.bb.tile([P, D], f32, tag="bb")
    embwb = g.rowp.tile([1, 2, D], f32, tag="row")
    nc.sync.dma_start(out=embwb[:, 0, :], in_=g.emb_w[:])
    nc.gpsimd.partition_broadcast(lnw_b[:], embwb[:, 0, :])
    nc.sync.dma_start(out=embwb[:, 1, :], in_=g.emb_b[:])
    nc.gpsimd.partition_broadcast(lnb_b[:], embwb[:, 1, :])

    hT = g.act.tile([P, KC, N], g.adt, tag="act", name="hT_emb")
    for tt in range(NT):
        _embed_tile(g, hT, tt, lnw_b, lnb_b)
    return hT


def _embed_tile(g, hT, tt, lnw_b, lnb_b):
    nc = g.nc
    idx = g.small.tile([P, 1], i32, tag="idx")
    nc.sync.dma_start(out=idx[:], in_=g.ids[tt * P:(tt + 1) * P, :])
    gt = g.wp.tile([P, D], f32, tag="w")
    nc.gpsimd.indirect_dma_start(
        out=gt[:], out_offset=None, in_=g.word_emb[:],
        in_offset=bass.IndirectOffsetOnAxis(ap=idx[:, :1], axis=0),
    )
    pt = g.wp.tile([P, D], f32, tag="w")
    nc.sync.dma_start(out=pt[:], in_=g.pos_type[(tt % 4) * P:(tt % 4 + 1) * P, :])
    htok = g.emb.tile([P, D], f32, tag="emb")
    nc.vector.tensor_add(htok[:], gt[:], pt[:])
    xr = htok[:].rearrange("p (s f) -> p s f", f=256)
    stats = g.small.tile([P, 3, 6], f32, tag="bnst")
    for sgi in range(3):
        nc.vector.bn_stats(out=stats[:, sgi, :], in_=xr[:, sgi, :])
    mv = g.small.tile([P, 2], f32, tag="bnmv")
    nc.vector.bn_aggr(out=mv[:], in_=stats[:])
    sd = g.small.tile([P, 1], f32, tag="sd")
    nc.scalar.activation(sd[:], mv[:, 1:2], AF.Sqrt, bias=g.epsT[:, 0:1], scale=1.0)
    nc.vector.reciprocal(sd[:], sd[:])
    nc.vector.tensor_scalar(
        out=htok[:], in0=htok[:],
        scalar1=mv[:, 0:1], scalar2=sd[:, 0:1],
        op0=ALU.subtract, op1=ALU.mult,
    )
    nc.vector.tensor_mul(htok[:], htok[:], lnw_b[:])
    nc.vector.tensor_add(htok[:], htok[:], lnb_b[:])
    for k in range(KC):
        ps = g.pp.tile([P, 512], f32, tag="pp")
        nc.tensor.transpose(ps[:, 0:P], htok[:, k * P:(k + 1) * P], g.ident[:])
        nc.vector.tensor_copy(hT[:, k, tt * P:(tt + 1) * P], ps[:, 0:P])


def _layer(g, l, hT, wk):
    nc = g.nc
    bias = g.biasp.tile([P, 8, KC], f32, tag="b6", name=f"bias{l}")
    for j, dram in enumerate((g.bqr, g.bkr, g.bor, g.bf2r,
                              g.ln1wr, g.ln1br, g.ln2wr, g.ln2br)):
        nc.sync.dma_start(out=bias[:, j, :], in_=dram[l])
    bqt, bkt, bot, bf2t = (bias[:, j, :] for j in range(4))
    l1w, l1b, l2w, l2b = (bias[:, j, :] for j in range(4, 8))
    bf1t = g.biasp.tile([P, FC], f32, tag="b24")
    nc.sync.dma_start(out=bf1t[:], in_=g.bf1r[l])
    bvrow = g.rowp.tile([1, D], f32, tag="row")
    nc.sync.dma_start(out=bvrow[:], in_=g.bv_row[l])
    bvb = g.bb.tile([P, D], f32, tag="bb")
    nc.gpsimd.partition_broadcast(bvb[:], bvrow[:])

    # ---- K projection (preloaded weight, chunk-outer so the LN2 tail
    # hides) interleaved with the matching V-projection token tiles and the
    # first head-block's Q/attention so the PE always has chunk-ready work ----
    kT = g.act.tile([P, KC, N], g.adt, tag="act", name=f"kT{l}")
    aT = g.act.tile([P, KC, N], g.adt, tag="act", name=f"aT{l}")
    wvf = g.wvp.tile([P, KC, D], g.adt, tag="wv", name=f"wv{l}")
    nc.sync.dma_start(out=wvf[:], in_=g.Wv[l].rearrange("(kc p) n -> p kc n", p=P))
    wqv = g.Wq[l].rearrange("(kc p) n -> p kc n", p=P)
    wmbq = g.wp.tile([P, KC, P], g.adt, tag="w", name=f"wq0_{l}")
    nc.sync.dma_start(out=wmbq[:], in_=wqv[:, :, 0:P])
    v = g.vp.tile([P, H, NT, HD + 1], g.pdt, tag="v")
    nc.vector.tensor_copy(v[:, :, :, HD:HD + 1],
                          g.ones[:].to_broadcast((P, H, NT, 1)))
    qm0 = []
    for c in range(2):
        cs = slice(c * 512, (c + 1) * 512)
        for m in range(KC):
            ps = g.pp.tile([P, 512], f32, tag="pp")
            for k in range(KC):
                nc.tensor.matmul(
                    ps[:], lhsT=wk[:, k, m * P:(m + 1) * P],
                    rhs=hT[:, k, cs],
                    start=(k == 0), stop=(k == KC - 1),
                )
            nc.vector.tensor_scalar_add(kT[:, m, cs], ps[:], bkt[:, m:m + 1])
        for tt in range(4 * c, 4 * c + 4):
            _vproj_tile(g, tt, hT, wvf, bvb, v)
        # Q projection for head-block 0, this chunk, then its attention
        qm = g.qmp.tile([P, 512], g.adt, tag="qm", name=f"qm0{c}")
        ps = g.pp.tile([P, 512], f32, tag="pp")
        for k in range(KC):
            nc.tensor.matmul(
                ps[:], lhsT=wmbq[:, k, :], rhs=hT[:, k, cs],
                start=(k == 0), stop=(k == KC - 1),
            )
        nc.vector.tensor_scalar_add(qm[:], ps[:], bqt[:, 0:1])
        qm0.append(qm)
        for hh in range(2):
            _attn_head_seq(g, 0, hh, c, kT, qm, v, aT)
    del qm0

    # ---- attention, remaining head blocks ----
    for mcb in range(1, KC):
        _attn_block(g, l, mcb, hT, kT, v, aT, bqt)

    # ---- O projection + residual + LN1 in-place (c-outer) ----
    x = g.act.tile([P, KC, N], g.adt, tag="act", name=f"x{l}")
    wov = g.Wo[l].rearrange("(kc p) n -> p kc n", p=P)
    for c in range(2):
        cs = slice(c * 512, (c + 1) * 512)
        for mp in range(KC // 2):
            wmb = g.wp.tile([P, KC, 2 * P], g.adt, tag="w")
            nc.sync.dma_start(
                out=wmb[:], in_=wov[:, :, 2 * mp * P:(2 * mp + 2) * P])
            for mi in range(2):
                m = 2 * mp + mi
                ps = g.pp.tile([P, 512], f32, tag="pp")
                for k in range(KC):
                    nc.tensor.matmul(
                        ps[:], lhsT=wmb[:, k, mi * P:(mi + 1) * P],
                        rhs=aT[:, k, cs],
                        start=(k == 0), stop=(k == KC - 1),
                    )
                if not g.zb:
                    nc.vector.tensor_scalar_add(ps[:], ps[:], bot[:, m:m + 1])
                nc.vector.tensor_tensor(
                    out=x[:, m, cs], in0=ps[:], in1=hT[:, m, cs], op=ALU.add)
        _ln_chunk(g, x, l1w, l1b, c)
    h1 = x  # normalized in place

    # ---- FFN (token-chunked, c-outer) + LN2 in-place ----
    x2 = g.act.tile([P, KC, N], g.adt, tag="act", name=f"x2{l}")
    for c in range(2):
        _ffn_chunk(g, l, c, h1, x2, bf1t, bf2t)
        _ln_chunk(g, x2, l2w, l2b, c)

    # prefetch next layer's K weight (issued after this layer's DMAs)
    if l + 1 < g.L:
        wk_n = g.wfp.tile([P, KC, D], f32r, tag="wfull", name=f"wk{l + 1}")
        nc.sync.dma_start(
            out=wk_n[:], in_=g.Wk[l + 1].rearrange("(kc p) n -> p kc n", p=P))
    else:
        wk_n = None
    return x2, wk_n


def _vproj_tile(g, tt, hT, wvf, bvb, v):
    nc = g.nc
    for (cs, cl) in ((0, 512), (512, 256)):
        ps = g.pp.tile([P, 512], f32, tag="pp")
        for k in range(KC):
            nc.tensor.matmul(
                ps[:, :cl], lhsT=hT[:, k, tt * P:(tt + 1) * P],
                rhs=wvf[:, k, cs:cs + cl],
                start=(k == 0), stop=(k == KC - 1),
            )
        nh = cl // HD
        h0 = cs // HD
        nc.vector.tensor_tensor(
            out=v[:, h0:h0 + nh, tt, 0:HD],
            in0=ps[:, :cl].rearrange("p (h d) -> p h d", d=HD),
            in1=bvb[:, cs:cs + cl].rearrange("p (h d) -> p h d", d=HD),
            op=ALU.add,
        )


def _attn_block(g, l, mcb, hT, kT, v, aT, bqt):
    """Q projection for feature block mcb (heads 2*mcb, 2*mcb+1) then attention."""
    nc = g.nc
    wqv = g.Wq[l].rearrange("(kc p) n -> p kc n", p=P)
    wmb = g.wp.tile([P, KC, P], g.adt, tag="w")
    nc.sync.dma_start(out=wmb[:], in_=wqv[:, :, mcb * P:(mcb + 1) * P])
    qms = []
    for c in range(2):
        cs = slice(c * 512, (c + 1) * 512)
        qm = g.qmp.tile([P, 512], g.adt, tag="qm", name=f"qm{c}")
        ps = g.pp.tile([P, 512], f32, tag="pp")
        for k in range(KC):
            nc.tensor.matmul(
                ps[:], lhsT=wmb[:, k, :], rhs=hT[:, k, cs],
                start=(k == 0), stop=(k == KC - 1),
            )
        nc.vector.tensor_scalar_add(qm[:], ps[:], bqt[:, mcb:mcb + 1])
        qms.append(qm)
    for hh in range(2):
        for s in range(2):
            _attn_head_seq(g, mcb, hh, s, kT, qms[s], v, aT)


def _attn_head_seq(g, mcb, hh, s, kT, qm, v, aT):
    nc = g.nc
    h = 2 * mcb + hh
    et = [None, None]
    for half in range(2):
        et[half] = g.expp.tile([P, 2, S], g.pdt, tag="exp", name=f"et{half}")
        for i in range(2):
            ck = half * 2 + i
            sc = g.scp.tile([P, S], f32, tag="sc")
            nc.tensor.matmul(
                sc[:],
                lhsT=kT[hh * HD:(hh + 1) * HD, mcb,
                        s * S + ck * P:s * S + (ck + 1) * P],
                rhs=qm[hh * HD:(hh + 1) * HD, :],
                start=True, stop=True,
            )
            nc.scalar.activation(et[half][:, i, :], sc[:], AF.Exp, scale=0.125)
    av = g.avp.tile([P, S], f32, tag="av")
    for ck in range(4):
        nc.tensor.matmul(
            av[0:HD + 1, :],
            lhsT=v[:, h, s * 4 + ck, :],
            rhs=et[ck // 2][:, ck % 2, :],
            start=(ck == 0), stop=(ck == 3),
        )
    dinv = g.small.tile([1, S], f32, tag="dinv")
    nc.vector.reciprocal(dinv[:], av[HD:HD + 1, :])
    dib = g.dvp.tile([HD, S], f32, tag="dv")
    nc.gpsimd.partition_broadcast(dib[:], dinv[:])
    nc.vector.tensor_tensor(
        out=aT[hh * HD:(hh + 1) * HD, mcb, s * S:(s + 1) * S],
        in0=av[0:HD, :], in1=dib[:], op=ALU.mult,
    )


def _ffn_chunk(g, l, c, h1, x2, bf1t, bf2t):
    nc = g.nc
    f1v = g.Wf1[l].rearrange("(kc p) n -> p kc n", p=P)
    f2v = g.Wf2[l].rearrange("(kc p) n -> p kc n", p=P)
    cs = slice(c * 512, (c + 1) * 512)
    ffT = g.ffp.tile([P, FC, 512], g.adt, tag="ff")
    for mp in range(FC // 2):
        wmb = g.wp.tile([P, KC, 2 * P], g.adt, tag="w")
        nc.sync.dma_start(
            out=wmb[:], in_=f1v[:, :, 2 * mp * P:(2 * mp + 2) * P])
        for mi in range(2):
            m = 2 * mp + mi
            ps = g.pp.tile([P, 512], f32, tag="pp")
            for k in range(KC):
                nc.tensor.matmul(
                    ps[:], lhsT=wmb[:, k, mi * P:(mi + 1) * P],
                    rhs=h1[:, k, cs],
                    start=(k == 0), stop=(k == KC - 1),
                )
            nc.scalar.activation(
                ffT[:, m, :], ps[:], g.act_fn, bias=bf1t[:, m:m + 1], scale=1.0)
    for m in range(KC):
        ps = g.pp.tile([P, 512], f32, tag="pp")
        for khalf in range(2):
            wmb2 = g.wp.tile([P, 12, P], g.adt, tag="w")
            nc.sync.dma_start(
                out=wmb2[:],
                in_=f2v[:, khalf * 12:(khalf + 1) * 12, m * P:(m + 1) * P])
            for kk in range(12):
                k = khalf * 12 + kk
                nc.tensor.matmul(
                    ps[:], lhsT=wmb2[:, kk, :], rhs=ffT[:, k, :],
                    start=(k == 0), stop=(k == FC - 1),
                )
        if not g.zb:
            nc.vector.tensor_scalar_add(ps[:], ps[:], bf2t[:, m:m + 1])
        nc.vector.tensor_tensor(
            out=x2[:, m, cs], in0=ps[:], in1=h1[:, m, cs], op=ALU.add)


def _ln_chunk(g, x, w, b, c):
    """In-place LayerNorm of feature-major x for token chunk c."""
    nc = g.nc
    cs = slice(c * 512, (c + 1) * 512)
    rows = g.rowsp.tile([1, 3, 512], f32, tag="rows")
    mrow, qrow, m2 = rows[:, 0, :], rows[:, 1, :], rows[:, 2, :]
    pS = g.lnp.tile([1, 512], f32, tag="ln")
    for k in range(KC):
        nc.tensor.matmul(pS[:], lhsT=g.onesr[:], rhs=x[:, k, cs],
                         start=(k == 0), stop=(k == KC - 1))
    nc.vector.tensor_scalar_mul(mrow, pS[:], 1.0 / D)
    pQ = g.lnp.tile([1, 512], f32, tag="ln")
    for k in range(KC):
        sq = g.qmp.tile([P, 512], g.adt, tag="qm")
        nc.vector.tensor_mul(sq[:], x[:, k, cs], x[:, k, cs])
        nc.tensor.matmul(pQ[:], lhsT=g.onesr[:], rhs=sq[:],
                         start=(k == 0), stop=(k == KC - 1))
    nc.vector.tensor_scalar_mul(qrow, pQ[:], 1.0 / D)
    # var = E[x^2] - mean^2 ; A = 1/sqrt(var+eps) ; B = -mean*A
    nc.vector.tensor_mul(m2, mrow, mrow)
    nc.vector.tensor_tensor(out=qrow, in0=qrow, in1=m2, op=ALU.subtract)
    nc.scalar.activation(qrow, qrow, AF.Sqrt, bias=g.epsT[0:1, 0:1], scale=1.0)
    nc.vector.reciprocal(qrow, qrow)
    nc.vector.tensor_mul(m2, mrow, qrow)
    nc.vector.tensor_scalar_mul(m2, m2, -1.0)
    A = g.qmp.tile([P, 512], f32, tag="qm", name="lnA")
    B = g.qmp.tile([P, 512], f32, tag="qm", name="lnB")
    nc.gpsimd.partition_broadcast(A[:], qrow)
    nc.gpsimd.partition_broadcast(B[:], m2)
    for k in range(KC):
        nc.vector.tensor_mul(x[:, k, cs], x[:, k, cs], A[:])
        nc.gpsimd.tensor_add(x[:, k, cs], x[:, k, cs], B[:])
        if not g.zb:
            nc.gpsimd.tensor_scalar(
                out=x[:, k, cs], in0=x[:, k, cs],
                scalar1=w[:, k:k + 1], scalar2=b[:, k:k + 1],
                op0=ALU.mult, op1=ALU.add)


# ======================= host-side prep / sharding =======================


def _r6(a, L, nchunk):
    # [L, D_or_F] -> [L, P, nchunk] with feature f = k*128 + p
    Ld = np.asarray(a)[:L]
    return np.ascontiguousarray(
        Ld.reshape(L, nchunk, P).transpose(0, 2, 1)).astype(np.float32)


def prep_shared(inputs, L=12, use_f32r=True):
    del use_f32r
    import ml_dtypes
    f = lambda x: np.ascontiguousarray(np.asarray(x, dtype=np.float32))
    w = {
        "word_emb": f(inputs["word_emb"]),
        "pos_type": f(np.asarray(inputs["pos_emb"])[:S] +
                      np.asarray(inputs["type_emb"])[0][None, :]),
        "emb_w": f(inputs["emb_ln_w"]).reshape(1, D),
        "emb_b": f(inputs["emb_ln_b"]).reshape(1, D),
        "Wq": f(inputs["Wq"][:L]), "Wk": f(inputs["Wk"][:L]),
        "Wv": f(inputs["Wv"][:L]), "Wo": f(inputs["Wo"][:L]),
        "Wf1": f(inputs["Wf1"][:L]), "Wf2": f(inputs["Wf2"][:L]),
        "bqr": _r6(inputs["bq"], L, KC),
        "bkr": _r6(inputs["bk"], L, KC),
        "bv_row": f(inputs["bv"][:L]).reshape(L, 1, D),
        "bor": _r6(inputs["bo"], L, KC),
        "bf1r": _r6(inputs["bf1"], L, FC),
        "bf2r": _r6(inputs["bf2"], L, KC),
        "ln1wr": _r6(inputs["ln1_w"], L, KC),
        "ln1br": _r6(inputs["ln1_b"], L, KC),
        "ln2wr": _r6(inputs["ln2_w"], L, KC),
        "ln2br": _r6(inputs["ln2_b"], L, KC),
        "onesd": np.ones((P, 1), np.float32).astype(ml_dtypes.bfloat16),
        "onesrd": np.ones((P, 1), np.float32),
    }
    return w


def core_ids_input(input_ids, core):
    return np.ascontiguousarray(
        np.asarray(input_ids)[2 * core:2 * core + 2].reshape(N, 1)).astype(np.int32)


def assemble_output(out_fm):
    # [KC, P, N] feature-major -> [2, S, D] token-major
    return np.ascontiguousarray(
        np.asarray(out_fm, dtype=np.float32).reshape(D, N).T).reshape(2, S, D)


_NC_CACHE = {}


def kernel(**inputs):
    from concourse.bass_utils import run_bass_kernel_spmd

    am = np.asarray(inputs["attention_mask"])
    assert (am == 1).all(), "kernel specialized for all-ones attention_mask"

    zb = all(
        not np.any(np.asarray(inputs[k]))
        for k in ("bq", "bk", "bv", "bo", "bf1", "bf2", "ln1_b", "ln2_b")
    ) and all(
        np.all(np.asarray(inputs[k]) == 1.0) for k in ("ln1_w", "ln2_w")
    )
    if _NC_CACHE.get("zb") != zb:
        _NC_CACHE["nc"] = build_nc(L=12, zero_bias=zb)
        _NC_CACHE["zb"] = zb
    nc = _NC_CACHE["nc"]

    shared = prep_shared(inputs, L=12)
    in_maps = []
    for core in range(8):
        m = dict(shared)
        m["ids"] = core_ids_input(inputs["input_ids"], core)
        in_maps.append(m)

    res = run_bass_kernel_spmd(nc, in_maps, list(range(8)), trace=False)
    out = np.concatenate(
        [assemble_output(res.results[c]["out_fm"]) for c in range(8)], axis=0)
    return out.astype(np.float32)
